# revision 49
# baseline (speedup 1.0000x reference)
"""Trainium2 Bass kernel: batched dense attention (softmax(Q S^T / sqrt(H)) S).

Full problem: query [4, 4096, 1024], source [4, 4096, 1024] (source doubles
as values), output [4, 4096, 1024], all float32.

Sharding: pure data parallel over 8 NeuronCores -- core c handles batch
c//2, query rows (c%2)*2048 ... +2048 with the full source for that batch
replicated to the core host-side.  No collectives are needed.

The host pre-casts Q and S to bf16 and ALSO ships pre-transposed copies
(Q^T and S^T), so the device never runs a single PE transpose for operand
layout.  The PE instruction stream is almost exactly the 2 x 1024 big
matmuls the math requires (~443 us at the sustained 2.4 GHz clock).

Per-core kernel ("transposed-P1" flash attention, bf16 matmuls, f32 PSUM):
  per 512-wide query chunk c (4 per core):
    P1: for each 128-row source tile st (32): accumulate over 8 h-chunks
        L^T[st][s=128, q=512] = S^T-block.T @ Q^T-chunk        (PSUM f32)
        ACT exp(scale*L^T) -> W^T st-block in SBUF bf16 (no max subtract;
        logits/32 ~ N(0,1) so exp is tame)
        DVE accumulates W^T blocks into acc[128, 512] f32
    denominator: 4 f32 PE transposes of acc blocks -> [q=128, p=128],
        DVE reduce_sum + reciprocal -> rinv[qt][128, 1] (one per q-tile)
    P2: for each q-tile (4) x h-half (2): accumulate over 32 source tiles
        O[q=128, h=512] = W^T-block.T @ S_nat                  (PSUM f32)
        DVE scales by rinv -> bf16 out tile -> DMA to DRAM

Scheduling around the ~190 GB/s aggregate HBM load bandwidth:
  - staging order: S^T blocks (sb-major) with chunk-1's Q^T slotted after
    sb1, then S natural -- all on the sync queue; chunk-0 Q^T alone on the
    scalar queue.
  - chunk 1's first 16 P1 chains run BEFORE P2 of chunk 0 (second W^T
    buffer), deferring P2-c0 past the point where S natural has landed.
  - outputs ride the sync queue (idle once staging is done); the late Q^T
    chunks ride the gpsimd queue, self-gated behind the first output DMAs.
"""

import math

import numpy as np

B, LQ, LS, H = 4, 4096, 4096, 1024
N_CORES = 8
Q_SPLIT = 2  # query-length split within each batch entry
LQ_SH = LQ // Q_SPLIT  # 2048 query rows per core

P = 128  # partitions
QC = 512  # query chunk width (moving-operand width for P1)
OC = 512  # output chunk width (h columns per P2 matmul)
N_WARM = 10  # junk matmuls covering DMA spin-up + PE p-state ramp at t=0


def _build(lq_sh, ls, h):
    """Build + compile the per-core Bass graph for shard shapes."""
    import concourse.bacc as bacc
    import concourse.mybir as mybir
    import concourse.tile as tile
    from concourse import masks

    f32 = mybir.dt.float32
    bf16 = mybir.dt.bfloat16

    n_qc = lq_sh // QC  # query chunks (4)
    n_qt = QC // P  # query tiles per chunk (4)
    n_st = ls // P  # source tiles (32)
    n_hc = h // P  # h chunks (contraction tiles for P1) (8)
    n_oc = h // OC  # output chunks (2)
    n_sb = ls // QC  # S^T column staging blocks (8)
    n_defer = 16  # chunk-1 P1 chains run ahead of P2 chunk 0
    scale = 1.0 / math.sqrt(h)

    nc = bacc.Bacc(
        "TRN2",
        target_bir_lowering=False,
        debug=False,
        num_devices=N_CORES,
    )
    qT_h = nc.dram_tensor("query_T", [h, lq_sh], bf16, kind="ExternalInput")
    sT_h = nc.dram_tensor("source_T", [h, ls], bf16, kind="ExternalInput")
    s_h = nc.dram_tensor("source_input", [ls, h], bf16, kind="ExternalInput")
    o_h = nc.dram_tensor("out", [lq_sh, h], bf16, kind="ExternalOutput")
    qT_ap, sT_ap, s_ap, o_ap = qT_h.ap(), sT_h.ap(), s_h.ap(), o_h.ap()

    with tile.TileContext(nc) as tc:
        from contextlib import ExitStack

        with ExitStack() as ctx:
            ident_pool = ctx.enter_context(tc.tile_pool(name="ident", bufs=1))
            ident_f32 = ident_pool.tile([P, P], f32)
            masks.make_identity(nc, ident_f32[:])

            # PE warmup: junk matmuls cover the ~9 us DMA pipeline spin-up
            # at t=0 and ramp the PE p-state before real chains arrive.
            warm_pool = ctx.enter_context(tc.tile_pool(name="warm", bufs=1))
            warm_w = warm_pool.tile([P, P], bf16)
            warm_x = warm_pool.tile([P, QC], bf16)
            nc.vector.memset(warm_w[:], 0.0)
            nc.vector.memset(warm_x[:], 0.0)
            psum_lg = ctx.enter_context(
                tc.tile_pool(name="psum_lg", bufs=3, space="PSUM")
            )
            wp = psum_lg.tile([P, QC], f32, tag="lg", name="warmpsum")
            for _ in range(N_WARM):
                nc.tensor.matmul(wp[:], warm_w[:], warm_x[:], start=True, stop=True)

            persist = ctx.enter_context(tc.tile_pool(name="persist", bufs=1))
            # S^T as ONE tile, hc-major columns: block hc at cols [hc*ls, +ls).
            s_T_all = persist.tile([P, n_hc * ls], bf16, tag="sT", name="sT")
            s_nat = persist.tile([P, n_st * h], bf16)

            def sT_sl(hc, st):
                return s_T_all[:, hc * ls + st * P : hc * ls + (st + 1) * P]
            # W^T buffers: main holds the active chunk's 32 st-blocks; the
            # small one holds chunk-1's first n_defer blocks so its P1 can
            # run ahead of P2 chunk 0.
            wT = persist.tile([P, n_st * QC], bf16, tag="wT", name="wT")
            wT_b = persist.tile([P, n_defer * QC], bf16, tag="wTb", name="wTb")
            acc_a = persist.tile([P, QC], f32, tag="acca", name="acca")
            acc_b = persist.tile([P, QC], f32, tag="accb", name="accb")

            qT_pool = ctx.enter_context(tc.tile_pool(name="qT", bufs=2))

            # Each DMA_DIRECT2D costs ~650 ns of engine issue time, so
            # staging uses BATCHED 3D-AP transfers: one 1 MB DMA per S^T
            # column block / Q^T chunk (dst [p, hc, c] <- src [hc, p, c]),
            # four 2 MB DMAs for S natural.
            def load_qT(c, eng):
                t = qT_pool.tile([P, n_hc * QC], bf16, tag="qTc")
                eng.dma_start(
                    t[:].rearrange("p (hc c) -> p hc c", c=QC),
                    qT_ap[:, c * QC : (c + 1) * QC].rearrange(
                        "(hc p) c -> p hc c", p=P
                    ),
                )
                return t

            sT3 = s_T_all[:].rearrange("p (hc s) -> p hc s", s=ls)

            # Chunk-0 Q^T alone on the scalar queue: together with sb0 it is
            # the ~2 MB of head-critical data.
            qT_tiles = {0: load_qT(0, nc.scalar)}
            for sb in range(n_sb):
                nc.sync.dma_start(
                    sT3[:, :, sb * QC : (sb + 1) * QC],
                    sT_ap[:, sb * QC : (sb + 1) * QC].rearrange(
                        "(hc p) c -> p hc c", p=P
                    ),
                )
                if sb == 2:
                    qT_tiles[1] = load_qT(1, nc.sync)
            sn3 = s_nat[:].rearrange("p (st h) -> p st h", h=h)
            for g in range(4):
                nc.sync.dma_start(
                    sn3[:, g * 8 : (g + 1) * 8, :],
                    s_ap[g * 8 * P : (g + 1) * 8 * P, :].rearrange(
                        "(st p) h -> p st h", p=P
                    ),
                )

            r_pool = ctx.enter_context(tc.tile_pool(name="racc", bufs=10))
            psum_tr = ctx.enter_context(
                tc.tile_pool(name="psum_tr", bufs=1, space="PSUM")
            )
            psum_o = ctx.enter_context(
                tc.tile_pool(name="psum_o", bufs=3, space="PSUM")
            )
            osb_pool = ctx.enter_context(tc.tile_pool(name="osb", bufs=2))

            def wt_ap(c, st):
                if c == 1 and st < n_defer:
                    return wT_b[:, st * QC : (st + 1) * QC]
                return wT[:, st * QC : (st + 1) * QC]

            def p1_chain(c, qTc, acc_t, st):
                lg = psum_lg.tile([P, QC], f32, tag="lg")
                for hc in range(n_hc):
                    nc.tensor.matmul(
                        lg[:],
                        sT_sl(hc, st),
                        qTc[:, hc * QC : (hc + 1) * QC],
                        start=(hc == 0),
                        stop=(hc == n_hc - 1),
                    )
                nc.scalar.activation(
                    wt_ap(c, st),
                    lg[:],
                    mybir.ActivationFunctionType.Exp,
                    scale=scale,
                )
                if st == 0:
                    nc.vector.tensor_copy(acc_t[:], wt_ap(c, 0))
                else:
                    nc.vector.tensor_add(acc_t[:], acc_t[:], wt_ap(c, st))

            def p1_chains(c, qTc, acc_t, sts):
                for st in sts:
                    p1_chain(c, qTc, acc_t, st)

            def p2_chunk(c, acc_t):
                rinv = []
                dpt = psum_tr.tile([P, QC], f32, tag="dtr")

                def emit_denom():
                    for j in range(n_qt):
                        nc.tensor.transpose(
                            dpt[:, j * P : (j + 1) * P],
                            acc_t[:, j * P : (j + 1) * P],
                            ident_f32[:],
                        )
                    for j in range(n_qt):
                        den = r_pool.tile([P, 1], f32, tag="den")
                        nc.vector.reduce_sum(
                            den[:], dpt[:, j * P : (j + 1) * P],
                            axis=mybir.AxisListType.X,
                        )
                        ri = r_pool.tile([P, 1], f32, tag="rinv")
                        nc.vector.reciprocal(ri[:], den[:])
                        rinv.append(ri)

                for qt in range(n_qt):
                    ob = osb_pool.tile([P, h], bf16, tag="ob")
                    for oci in range(n_oc):
                        op = psum_o.tile([P, OC], f32, tag="opsum")
                        for st in range(n_st):
                            nc.tensor.matmul(
                                op[:],
                                wt_ap(c, st)[:, qt * P : (qt + 1) * P],
                                s_nat[:, st * h + oci * OC : st * h + (oci + 1) * OC],
                                start=(st == 0),
                                stop=(st == n_st - 1),
                            )
                        if qt == 0 and oci == 0:
                            # denominator PE work slotted here so the PE
                            # never waits on the exp/acc tail of P1.
                            emit_denom()
                        # The very last output is split in two halves with
                        # the DMAs on different queues so the tail is one
                        # half-scale + one DMA deep instead of four.
                        last = c == 3 and qt == n_qt - 1 and oci == n_oc - 1
                        strips = 2 if last else 1
                        sw = OC // strips
                        for si in range(strips):
                            lo = oci * OC + si * sw
                            nc.vector.tensor_scalar_mul(
                                ob[:, lo : lo + sw], op[:, si * sw : (si + 1) * sw],
                                rinv[qt][:],
                            )
                            eng = nc.scalar if (last and si == 0) else nc.sync
                            eng.dma_start(
                                o_ap[
                                    c * QC + qt * P : c * QC + (qt + 1) * P,
                                    lo : lo + sw,
                                ],
                                ob[:, lo : lo + sw],
                            )

            # c0 P1 | c1 P1 (first n_defer chains) | c0 P2 | c1 P1 rest |
            # c1 P2 | c2 P1 | c2 P2 | c3 P1 | c3 P2
            qT0 = qT_tiles.pop(0)
            qT1 = qT_tiles.pop(1)
            p1_chains(0, qT0, acc_a, range(n_st))
            p1_chains(1, qT1, acc_b, range(n_defer))
            p2_chunk(0, acc_a)
            qT_tiles[2] = load_qT(2, nc.gpsimd)
            p1_chains(1, qT1, acc_b, range(n_defer, n_st))
            p2_chunk(1, acc_b)
            qT_tiles[3] = load_qT(3, nc.gpsimd)
            for c in (2, 3):
                qTc = qT_tiles.pop(c)
                acc_t = acc_a if c == 2 else acc_b
                p1_chains(c, qTc, acc_t, range(n_st))
                p2_chunk(c, acc_t)

    nc.compile()
    return nc


_cached_nc = None


def _get_nc():
    global _cached_nc
    if _cached_nc is None:
        _cached_nc = _build(LQ_SH, LS, H)
    return _cached_nc


def _in_maps(query_input, source_input):
    import ml_dtypes

    bf16 = ml_dtypes.bfloat16
    q = np.asarray(query_input, dtype=np.float32).astype(bf16)
    s = np.asarray(source_input, dtype=np.float32).astype(bf16)
    assert q.shape == (B, LQ, H) and s.shape == (B, LS, H)
    in_maps = []
    per_b = {}
    for b in range(B):
        sT = np.ascontiguousarray(s[b].T)
        qT = np.ascontiguousarray(q[b].T)
        per_b[b] = (np.ascontiguousarray(s[b]), sT, qT)
    for c in range(N_CORES):
        b, qh = divmod(c, Q_SPLIT)
        s_nat, sT, qT = per_b[b]
        in_maps.append(
            {
                "query_T": np.ascontiguousarray(
                    qT[:, qh * LQ_SH : (qh + 1) * LQ_SH]
                ),
                "source_T": sT,
                "source_input": s_nat,
            }
        )
    return in_maps


def _gather(results):
    out = np.empty((B, LQ, H), dtype=np.float32)
    for c in range(N_CORES):
        b, qh = divmod(c, Q_SPLIT)
        out[b, qh * LQ_SH : (qh + 1) * LQ_SH, :] = results[c]["out"]
    return out


def kernel(query_input, source_input):
    from concourse.bass_utils import run_bass_kernel_spmd

    res = run_bass_kernel_spmd(
        _get_nc(),
        _in_maps(query_input, source_input),
        core_ids=list(range(N_CORES)),
    )
    return _gather(res.results)
